# revision 26
# baseline (speedup 1.0000x reference)
"""Trainium2 Bass kernel: GroupNorm + single-head self-attention block.

Reference computation (per batch b):
    xn = GroupNorm(x, 16 groups, eps=1e-5) * gamma + beta
    q/k/v = W @ xn + b          (1x1 conv == channel matmul), [C, N]
    S = (q^T k) / sqrt(C)       [N, N]
    A = softmax_j(S)
    O = v @ A^T                 [C, N]
    y = wo @ O + bo + x

Shapes: B=4, C=256, H=W=64 -> N=4096.

Sharding: 8 cores = 4 batches x 2 query-halves.  Each core receives the
full x[b] with its query half permuted to the front, computes xn / v
for all N keys (cheap, avoids any collectives) and runs attention for
its 2048 queries.  The device program is identical on all cores (SPMD).

Algebraic restructuring (host-side, exact):
  - S^T[j,i] = sum_c k[c,j] q[c,i] = xn^T WQK xn with WQK = wq^T wk
    folded on the host; the per-query bias term from bk shifts all
    scores of a query equally and is dropped (softmax-invariant), the
    bq term survives as bqk = wk^T bq.
  - wo is folded into v: out = wo (v A_n^T) = (WOV xn + wo bv) A_n^T
    with WOV = wo wv.  The attention-value matmul then directly
    produces the final projection.

Device algorithm (per core):
  - GroupNorm stats via bn_stats/bn_aggr per channel + PE matmul with a
    group-indicator matrix for the cross-partition (channel) reduction.
  - qk = WQK^T xn + bqk for the 2048 local queries.
  - Scores computed TRANSPOSED per key-tile: S^T = xn^T qk, so both
    operands are natural [C, *] layouts (no transposes anywhere).
  - exp without max-subtraction (scores ~ N(0,1); fp32 exp is safe).
  - softmax denominator: ones-vector matmul over partitions on PE,
    broadcast back via a 0-stride-partition DMA, reciprocal on DVE.
  - out = v'^T A^T accumulated in PSUM, then *recip + residual on DVE.

Big matmuls run in float32r (full-rate fp32 PE mode).  fp32r operands
must be produced "rounded" by a compute engine, so every matmul input
tile is written by DVE/ACT with a float32r output dtype.
"""

import sys

sys.path.insert(0, "/opt/trn_rl_repo")

from contextlib import ExitStack

import numpy as np

import concourse.bacc as bacc
import concourse.bass as bass
import concourse.mybir as mybir
import concourse.tile as tile

B, C, H, W = 4, 256, 64, 64
N = H * W              # keys per batch
GROUPS = 16
EPS = 1e-5
NCORES = 8
QSPLIT = NCORES // B   # query shards per batch
NQ = N // QSPLIT       # queries per core
P = 128
CCH = C // P           # channel chunks (2)
IB = 512               # query block (one PSUM bank of f32)
NIB = NQ // IB         # query blocks per core
NJT = N // P           # key tiles (32)
GSZ = C // GROUPS      # channels per group (16)

F32 = mybir.dt.float32
F32R = mybir.dt.float32r
AF = mybir.ActivationFunctionType
OP = mybir.AluOpType


def build_nc(mm_dtype: str = "f32r"):
    """Emit the single-core SPMD program."""
    fp8_dr = mm_dtype.endswith("+fp8")
    base = mm_dtype.replace("+fp8", "")
    DTM = {"f32r": F32R, "bf16": mybir.dt.bfloat16, "f32": F32}[base]
    FP8 = mybir.dt.float8e4
    DTV = FP8 if fp8_dr else DTM   # dtype of the at / v' operands
    nc = bacc.Bacc()

    x_d = nc.declare_dram_parameter("x", [C, N], F32, isOutput=False)
    wqk_d = nc.declare_dram_parameter("wqk", [C, C], F32, isOutput=False)
    wovT_d = nc.declare_dram_parameter("wovT", [C, C], F32, isOutput=False)
    gamma_d = nc.declare_dram_parameter("gamma", [C], F32, isOutput=False)
    beta_d = nc.declare_dram_parameter("beta", [C], F32, isOutput=False)
    bqk_d = nc.declare_dram_parameter("bqk", [C], F32, isOutput=False)
    bvp_d = nc.declare_dram_parameter("bvp", [C], F32, isOutput=False)
    bo_d = nc.declare_dram_parameter("bo", [C], F32, isOutput=False)
    gind_d = nc.declare_dram_parameter("gind", [CCH, P, GROUPS], F32, isOutput=False)
    gindT_d = nc.declare_dram_parameter("gindT", [CCH, GROUPS, P], F32, isOutput=False)
    y_d = nc.declare_dram_parameter("y", [C, NQ], F32, isOutput=True)

    with tile.TileContext(nc) as tc, ExitStack() as ctx:
        const = ctx.enter_context(tc.tile_pool(name="const", bufs=1))
        data = ctx.enter_context(tc.tile_pool(name="data", bufs=1))

        # ---- weights: DMA to f32 staging, DVE-copy to fp32r tiles ----
        stage = ctx.enter_context(tc.tile_pool(name="stage", bufs=1))

        # fp32r lhsT free-dim counts must be even -> ones "column" is [P, 2]
        # (memset cannot emit fp32r; stage in f32 and DVE-copy to round)
        ones_f = const.tile([P, P], F32, name="ones_f")
        nc.vector.memset(ones_f, 1.0)
        ones_col2 = const.tile([P, 2], DTM, name="ones_col2")
        nc.vector.tensor_copy(ones_col2, ones_f[:, 0:2])
        ones_row_r = const.tile([1, P], DTM, name="ones_row_r")
        nc.vector.tensor_copy(ones_row_r, ones_f[0:1, :])
        if fp8_dr:
            # DoubleRow ones "column": [K, 2 pair-slices, M=16] -- the pair
            # dim stride must be 16B-aligned, so M is padded to 16
            ones_dr = const.tile([P, 2, 16], FP8, name="ones_dr")
            nc.vector.tensor_copy(
                ones_dr, ones_f[:, 0:32].rearrange("p (a b) -> p a b", a=2)
            )
            neg_ln16 = const.tile([P, 1], F32, name="neg_ln16")
            nc.vector.memset(neg_ln16, -2.772588722239781)  # -ln(16)
        # PE HAM warm-up scaffolding: the clock gate only reaches 2.4 GHz
        # after ~3.4us of sustained activity and re-throttles after an idle
        # window, so burn dummy matmuls during the DMA/GroupNorm prologue
        # (PE is otherwise idle there) and drip data-dependent "pings" so
        # the gate never sees an idle window before the real matmuls start.
        warm_src_f = const.tile([P, 512], F32, name="warm_src_f")
        nc.vector.memset(warm_src_f, 0.0)
        warm_src = const.tile([P, 512], DTM, name="warm_src")
        nc.vector.tensor_copy(warm_src, warm_src_f)
        def load_w(handle, nm):
            tiles = []
            for ch in range(CCH):
                s = stage.tile([P, C], F32, name=f"{nm}{ch}_s", tag=f"{nm}{ch}_s")
                nc.sync.dma_start(out=s, in_=handle[ch * P:(ch + 1) * P, :])
                t = const.tile([P, C], DTM, name=f"{nm}{ch}")
                nc.vector.tensor_copy(t, s)
                tiles.append(t)
            return tiles

        wqk = load_w(wqk_d, "wqk")      # [c, c'] chunks; lhsT for qk proj
        wovT = load_w(wovT_d, "wovT")   # [c', o] chunks; rhs for v' proj

        def load_vec(handle, nm):
            tiles = []
            for ch in range(CCH):
                t = const.tile([P, 1], F32, name=f"{nm}{ch}")
                nc.sync.dma_start(
                    out=t, in_=handle[ch * P:(ch + 1) * P].unsqueeze(1)
                )
                tiles.append(t)
            return tiles

        gamma = load_vec(gamma_d, "gamma")
        beta = load_vec(beta_d, "beta")
        bqk = load_vec(bqk_d, "bqk")
        bo = load_vec(bo_d, "bo")

        bvp_s = stage.tile([1, C], F32, name="bvp_s")
        nc.sync.dma_start(out=bvp_s, in_=bvp_d[:].unsqueeze(0))
        bvp_row = const.tile([1, C], DTM, name="bvp_row")
        nc.vector.tensor_copy(bvp_row, bvp_s)

        gind = []
        gindT = []
        for ch in range(CCH):
            gi = const.tile([P, GROUPS], F32, name=f"gind{ch}")
            nc.sync.dma_start(out=gi, in_=gind_d[ch])
            gind.append(gi)
            gt = const.tile([GROUPS, P], F32, name=f"gindT{ch}")
            nc.sync.dma_start(out=gt, in_=gindT_d[ch])
            gindT.append(gt)


        # ---- x in (staging pool released after GroupNorm) ----
        xn = data.tile([P, CCH, N], DTM, name="xn")
        resid = data.tile([P, CCH, NQ], F32, name="resid")


        with tc.tile_pool(name="xf_pool", bufs=1) as xf_pool, \
             tc.tile_pool(name="gn_psum", bufs=1, space="PSUM") as gn_psum, \
             tc.tile_pool(name="warm_psum", bufs=1, space="PSUM") as warm_psum, \
             tc.tile_pool(name="gn_sb", bufs=1) as gn_sb:
            warm_ps = warm_psum.tile([P, 512], F32, name="warm_ps")

            def warm(rhs=None, n=1):
                # M=2 keeps the HAM activity monitor fed at ~1/64th of the
                # PE-array power (wide bursts trip the firmware throttle)
                for _ in range(n):
                    nc.tensor.matmul(
                        warm_ps[:2, :512] if rhs is None else warm_ps[:2, :rhs.shape[-1]],
                        lhsT=ones_col2 if rhs is None else ones_f[:, 0:2],
                        rhs=warm_src if rhs is None else rhs,
                        start=True, stop=True, skip_group_check=True,
                    )

            warm(n=26)  # ~3.5us+ dense burst at t=0 -> gate opens early
            xf = xf_pool.tile([P, CCH, N], F32, name="xf")
            NS = N // 512  # bn_stats subgroups; DMA per subgroup to overlap
            for ch in range(CCH):
                for sg in range(NS):
                    nc.sync.dma_start(
                        out=xf[:, ch, sg * 512:(sg + 1) * 512],
                        in_=x_d[ch * P:(ch + 1) * P, sg * 512:(sg + 1) * 512],
                    )
            # ---- GroupNorm stats ----
            # chunk 0 on DVE (bn_stats), chunk 1 on the otherwise-idle ACT
            # engine via activation accum_out (sum and sum-of-squares)
            pc = []  # per-channel [mean, E[x^2]-ish] per chunk
            st6 = gn_sb.tile([P, NS, 6], F32, name="st6_0")
            for sg in range(NS):
                nc.vector.bn_stats(
                    out=st6[:, sg, :], in_=xf[:, 0, sg * 512:(sg + 1) * 512]
                )
                warm(rhs=st6[:, sg, :])
            mv = gn_sb.tile([P, 2], F32, name="mv0")
            nc.vector.bn_aggr(out=mv, in_=st6)
            pcs0 = gn_sb.tile([P, 2], F32, name="pcs0")
            nc.vector.tensor_copy(pcs0[:, 0:1], mv[:, 0:1])
            # pcs[:,1] = mean^2 + var  (-> group E[x^2] after averaging)
            msq = gn_sb.tile([P, 1], F32, name="msq0")
            nc.vector.tensor_mul(msq, mv[:, 0:1], mv[:, 0:1])
            nc.vector.tensor_add(pcs0[:, 1:2], mv[:, 1:2], msq)
            pc.append(pcs0)

            sum1 = gn_sb.tile([P, 1], F32, name="sum1")
            nc.scalar.activation(
                out=xf[:, 1, :], in_=xf[:, 1, :], func=AF.Identity,
                accum_out=sum1,
            )
            sq1 = gn_sb.tile([P, CCH, N // CCH], F32, name="sq1")
            ss1 = gn_sb.tile([P, 1], F32, name="ss1")
            nc.scalar.activation(
                out=sq1.rearrange("p a b -> p (a b)"), in_=xf[:, 1, :],
                func=AF.Square, accum_out=ss1,
            )
            pcs1 = gn_sb.tile([P, 2], F32, name="pcs1")
            nc.vector.tensor_scalar_mul(pcs1[:, 0:1], sum1, scalar1=1.0 / N)
            nc.vector.tensor_scalar_mul(pcs1[:, 1:2], ss1, scalar1=1.0 / N)
            pc.append(pcs1)

            # residual (+ bo) for the local query half (ACT is idle here)
            for ch in range(CCH):
                nc.scalar.activation(
                    out=resid[:, ch, :], in_=xf[:, ch, :NQ], func=AF.Identity,
                    bias=bo[ch], scale=1.0,
                )

            gs_ps = gn_psum.tile([GROUPS, 2], F32, name="gs_ps")
            for ch in range(CCH):
                nc.tensor.matmul(
                    gs_ps, lhsT=gind[ch], rhs=pc[ch],
                    start=(ch == 0), stop=(ch == CCH - 1),
                )
            # per-channel stats are already means -> average over the GSZ
            # channels of each group
            gs = gn_sb.tile([GROUPS, 2], F32, name="gs")
            nc.scalar.mul(gs, gs_ps, 1.0 / GSZ)
            gvar = gn_sb.tile([GROUPS, 1], F32, name="gvar")
            gmsq = gn_sb.tile([GROUPS, 1], F32, name="gmsq")
            nc.vector.tensor_mul(gmsq, gs[:, 0:1], gs[:, 0:1])
            nc.vector.tensor_sub(gvar, gs[:, 1:2], gmsq)
            # rstd = 1/sqrt(var+eps)
            gstd = gn_sb.tile([GROUPS, 1], F32, name="gstd")
            eps_t = gn_sb.tile([GROUPS, 1], F32, name="eps_t")
            nc.vector.memset(eps_t, EPS)
            nc.scalar.activation(
                out=gstd, in_=gvar, func=AF.Sqrt, bias=eps_t, scale=1.0
            )
            gmr = gn_sb.tile([GROUPS, 2], F32, name="gmr")
            nc.vector.tensor_copy(gmr[:, 0:1], gs[:, 0:1])
            nc.vector.reciprocal(gmr[:, 1:2], gstd)

            # broadcast group (mean, rstd) back to channels, build affine
            for ch in range(CCH):
                cb_ps = gn_psum.tile([P, 2], F32, name="cb_ps", tag="cb_ps")
                nc.tensor.matmul(cb_ps, lhsT=gindT[ch], rhs=gmr,
                                 start=True, stop=True)
                cb = gn_sb.tile([P, 2], F32, name=f"cb{ch}")
                nc.vector.tensor_copy(cb, cb_ps)
                scale = gn_sb.tile([P, 1], F32, name=f"scale{ch}")
                nc.vector.tensor_mul(scale, gamma[ch], cb[:, 1:2])
                shift = gn_sb.tile([P, 1], F32, name=f"shift{ch}")
                nc.vector.tensor_mul(shift, cb[:, 0:1], scale)
                nc.vector.tensor_sub(shift, beta[ch], shift)
                # xn = x * scale + shift (column blocks -> projections
                # on early columns can start while later ones convert)
                for xb in range(4):
                    xsl = slice(xb * (N // 4), (xb + 1) * (N // 4))
                    nc.vector.tensor_scalar(
                        out=xn[:, ch, xsl], in0=xf[:, ch, xsl],
                        scalar1=scale, scalar2=shift, op0=OP.mult, op1=OP.add,
                    )
                warm(rhs=cb)

        # ---- projections ----
        qk = data.tile([P, CCH, NQ], DTM, name="qk")    # WQK^T xn + bqk
        vT = data.tile([P, NJT, C], DTV, name="vT")     # (WOV xn)^T + wo bv

        with tc.tile_pool(name="pj_psum", bufs=3, space="PSUM") as pj_psum:
            # v'-bias row broadcast once: b_sb[j, o] = bvp[o]
            bps = pj_psum.tile([P, C], F32, name="bps", tag="vT_ps")
            nc.tensor.matmul(bps, lhsT=ones_row_r, rhs=bvp_row,
                             start=True, stop=True)
            b_sb = const.tile([P, C], F32, name="b_sb")
            nc.vector.tensor_copy(b_sb, bps)
            # qk[c', i] = sum_c WQK[c, c'] xn[c, i] + bqk[c']
            for oc in range(CCH):
                for it in range(NQ // 512):
                    ps = pj_psum.tile([P, 512], F32, name="qk_ps", tag="qk_ps")
                    for ch in range(CCH):
                        nc.tensor.matmul(
                            ps,
                            lhsT=wqk[ch][:, oc * P:(oc + 1) * P],
                            rhs=xn[:, ch, it * 512:(it + 1) * 512],
                            start=(ch == 0), stop=(ch == CCH - 1),
                        )
                    nc.vector.tensor_scalar_add(
                        qk[:, oc, it * 512:(it + 1) * 512], ps, scalar1=bqk[oc]
                    )
            # vT[j, o] = sum_c' xn[c', j] WOV[o, c'] + (wo bv)[o]
            for jt in range(NJT):
                ps = pj_psum.tile([P, C], F32, name="vT_ps", tag="vT_ps")
                for ch in range(CCH):
                    nc.tensor.matmul(
                        ps,
                        lhsT=xn[:, ch, jt * P:(jt + 1) * P],
                        rhs=wovT[ch],
                        start=(ch == 0), stop=(ch == CCH - 1),
                    )
                nc.vector.tensor_add(vT[:, jt, :], ps, b_sb)

        # ---- attention ----
        with tc.tile_pool(name="st_psum", bufs=2, space="PSUM") as st_psum, \
             tc.tile_pool(name="o_psum", bufs=1, space="PSUM") as o_psum, \
             tc.tile_pool(name="sm_psum", bufs=1, space="PSUM") as sm_psum, \
             tc.tile_pool(name="at_pool", bufs=6) as at_pool, \
             tc.tile_pool(name="fin", bufs=2) as fin:
            for ib in range(NIB):
                isl = slice(ib * IB, (ib + 1) * IB)
                sums_ps = sm_psum.tile(
                    [16 if fp8_dr else 2, IB], F32, name="sums_ps", tag="sums"
                )
                o_ps = [
                    o_psum.tile([P, IB], F32, name=f"o_ps{cc}", tag=f"o{cc}")
                    for cc in range(CCH)
                ]
                # Software-pipelined on key-tile PAIRS: the score PSUM
                # tile holds two key-tiles (2 banks) so ONE exp covers the
                # pair and writes the fp8 DoubleRow [K, 2, N] layout
                # directly.  DR matmuls consume the pair with a 1-pair lag
                # so their waits are pre-satisfied.
                if fp8_dr:
                    PLAG = 1
                    npair = NJT // 2
                    ats = {}
                    for p in range(npair + PLAG):
                        if p < npair:
                            stp = st_psum.tile([P, 2, IB], F32, name="stp", tag="st")
                            for m in range(2):
                                jt = 2 * p + m
                                jsl = slice(jt * P, (jt + 1) * P)
                                for ch in range(CCH):
                                    nc.tensor.matmul(
                                        stp[:, m, :],
                                        lhsT=xn[:, ch, jsl],
                                        rhs=qk[:, ch, isl],
                                        start=(ch == 0), stop=(ch == CCH - 1),
                                    )
                            atp = at_pool.tile([P, 2, IB], FP8, name="atp", tag="at")
                            # A^T = exp(S^T/16 - ln 16); the -ln16 keeps fp8e4
                            # in range and cancels in the normalization
                            nc.scalar.activation(
                                out=atp.rearrange("p a b -> p (a b)"),
                                in_=stp.rearrange("p a b -> p (a b)"),
                                func=AF.Exp, scale=1.0 / 16.0, bias=neg_ln16,
                            )
                            ats[p] = atp
                        if p >= PLAG:
                            pg = p - PLAG
                            atp = ats.pop(pg)
                            nc.tensor.matmul(
                                sums_ps, lhsT=ones_dr, rhs=atp,
                                start=(pg == 0), stop=(pg == npair - 1),
                                perf_mode=mybir.MatmulPerfMode.DoubleRow,
                            )
                            for cc in range(CCH):
                                nc.tensor.matmul(
                                    o_ps[cc],
                                    lhsT=vT[:, 2 * pg:2 * pg + 2,
                                            cc * P:(cc + 1) * P],
                                    rhs=atp,
                                    start=(pg == 0), stop=(pg == npair - 1),
                                    perf_mode=mybir.MatmulPerfMode.DoubleRow,
                                )
                else:
                    LAG = 2
                    ats = {}
                    for jt in range(NJT + LAG):
                        if jt < NJT:
                            jsl = slice(jt * P, (jt + 1) * P)
                            st = st_psum.tile([P, IB], F32, name="st", tag="st")
                            for ch in range(CCH):
                                nc.tensor.matmul(
                                    st,
                                    lhsT=xn[:, ch, jsl],
                                    rhs=qk[:, ch, isl],
                                    start=(ch == 0), stop=(ch == CCH - 1),
                                )
                            at = at_pool.tile([P, IB], DTM, name="at", tag="at")
                            nc.scalar.activation(
                                out=at, in_=st, func=AF.Exp, scale=1.0 / 16.0
                            )
                            ats[jt] = at
                        if jt >= LAG and (jt - LAG) % 2 == 1:
                            for g in (jt - LAG - 1, jt - LAG):
                                at_g = ats.pop(g)
                                nc.tensor.matmul(
                                    sums_ps, lhsT=ones_col2, rhs=at_g,
                                    start=(g == 0), stop=(g == NJT - 1),
                                )
                                for cc in range(CCH):
                                    nc.tensor.matmul(
                                        o_ps[cc],
                                        lhsT=vT[:, g, cc * P:(cc + 1) * P],
                                        rhs=at_g,
                                        start=(g == 0), stop=(g == NJT - 1),
                                    )

                # free the accumulators quickly so the next block's PE
                # matmuls don't wait on the normalization chain
                o_sb = []
                for cc in range(CCH):
                    t = fin.tile([P, IB], F32, name=f"o_sb{cc}", tag=f"osb{cc}")
                    nc.vector.tensor_copy(t, o_ps[cc])
                    o_sb.append(t)

                # denominator -> [128, IB] broadcast (PE) + reciprocal (DVE)
                sums_row = fin.tile([1, IB], F32, name="sums_row", tag="sums_row")
                nc.vector.tensor_copy(sums_row, sums_ps[0:1, :])
                rb_ps = sm_psum.tile([P, IB], F32, name="rb_ps", tag="rb")
                nc.tensor.matmul(rb_ps, lhsT=ones_f[0:1, :], rhs=sums_row,
                                 start=True, stop=True)
                rb = fin.tile([P, IB], F32, name="rb", tag="rbs")
                nc.vector.reciprocal(rb, rb_ps)

                for oc in range(CCH):
                    t = fin.tile([P, IB], F32, name="t_sb", tag="t_sb")
                    nc.vector.tensor_mul(t, o_sb[oc], rb)
                    out_sb = fin.tile([P, IB], F32, name="out_sb", tag="out_sb")
                    nc.vector.tensor_add(out_sb, t, resid[:, oc, isl])
                    nc.sync.dma_start(
                        out=y_d[oc * P:(oc + 1) * P, isl], in_=out_sb
                    )
    nc.finalize()
    return nc


_NC_CACHE = {}


def _get_nc(mm_dtype="f32r"):
    if mm_dtype not in _NC_CACHE:
        _NC_CACHE[mm_dtype] = build_nc(mm_dtype)
    return _NC_CACHE[mm_dtype]


def make_in_maps(inputs):
    """Shard full inputs into per-core input maps (host-side weight folding)."""
    x = np.asarray(inputs["x"], np.float32).reshape(B, C, N)
    gamma = np.asarray(inputs["gamma"], np.float32)
    beta = np.asarray(inputs["beta"], np.float32)
    wq = np.asarray(inputs["wq"], np.float64)
    bq = np.asarray(inputs["bq"], np.float64)
    wk = np.asarray(inputs["wk"], np.float64)
    wv = np.asarray(inputs["wv"], np.float64)
    bv = np.asarray(inputs["bv"], np.float64)
    wo = np.asarray(inputs["wo"], np.float64)
    bo = np.asarray(inputs["bo"], np.float32)

    # S^T = xn^T (wq^T wk) xn + (wk^T bq) broadcast over keys
    wqk = np.ascontiguousarray((wq.T @ wk).astype(np.float32))      # [c, c']
    bqk = (wk.T @ bq).astype(np.float32)                            # [c']
    # out = (wo wv xn + wo bv) A_n^T
    wovT = np.ascontiguousarray((wo @ wv).T.astype(np.float32))     # [c', o]
    bvp = (wo @ bv).astype(np.float32)                              # [o]

    gind = np.zeros((CCH, P, GROUPS), np.float32)
    for ch in range(CCH):
        for p in range(P):
            gind[ch, p, (ch * P + p) // GSZ] = 1.0
    gindT = np.ascontiguousarray(gind.transpose(0, 2, 1))

    shared = {
        "wqk": wqk, "wovT": wovT,
        "gamma": gamma, "beta": beta,
        "bqk": bqk, "bvp": bvp, "bo": bo,
        "gind": gind, "gindT": gindT,
    }
    in_maps = []
    for core in range(NCORES):
        b, h = divmod(core, QSPLIT)
        if h == 0:
            xc = x[b]
        else:
            xc = np.concatenate(
                [x[b][:, h * NQ:(h + 1) * NQ], x[b][:, :h * NQ],
                 x[b][:, (h + 1) * NQ:]], axis=1,
            )
        in_maps.append({"x": np.ascontiguousarray(xc), **shared})
    return in_maps


def gather_output(results):
    y = np.empty((B, C, N), np.float32)
    for core in range(NCORES):
        b, h = divmod(core, QSPLIT)
        y[b][:, h * NQ:(h + 1) * NQ] = results[core]["y"]
    return y.reshape(B, C, H, W)


def _run_traced(nc, in_maps, core_ids, tmpdir=None):
    """Replicates run_bass_kernel_spmd's axon trace branch; this image
    lacks antenv.axon_hooks, so drive the NTFF hook via ctypes directly."""
    import glob
    import tempfile

    import gauge.profiler
    from concourse import bass2jax
    from concourse._compat import FishPath
    from concourse.bass_utils import BassKernelResults, _process_ntff_profile
    from trn_agent_boot.trn_boot import _ntff_profile_via_ctypes

    hook = _ntff_profile_via_ctypes("/opt/axon/libaxon_pjrt.so")
    if tmpdir is None:
        tmpdir = tempfile.mkdtemp(prefix="bassprof_")
    if hook is None:
        results = bass2jax.run_bass_via_pjrt(nc, in_maps, n_cores=len(core_ids))
        return BassKernelResults(results, None, None, None)
    with hook(tmpdir, [0]):
        results = bass2jax.run_bass_via_pjrt(nc, in_maps, n_cores=len(core_ids))
    if not glob.glob(f"{tmpdir}/*_body*.ntff"):
        print(f"no NTFF produced in {tmpdir}")
        return BassKernelResults(results, None, None, None)
    profile = gauge.profiler.Profile(
        profile_path=FishPath(tmpdir),
        kernel_dev_mode=True,
        profile_on_exit=False,
        bass_kernel=nc.m,
        offline_processing=True,
        fname="*_body*",
        metadata={},
    )
    return _process_ntff_profile(
        profile, tmpdir, nc, core_ids, None, False, {}, False
    ).as_bass_kernel_results(results)


def run_spmd(inputs, trace=False, mm_dtype="f32r", tmpdir=None):
    from concourse.bass_utils import run_bass_kernel_spmd

    nc = _get_nc(mm_dtype)
    in_maps = make_in_maps(inputs)
    if trace:
        res = _run_traced(nc, in_maps, list(range(NCORES)), tmpdir=tmpdir)
    else:
        res = run_bass_kernel_spmd(nc, in_maps, list(range(NCORES)), trace=False)
    return gather_output(res.results), res


def kernel(**inputs) -> np.ndarray:
    out, _ = run_spmd(inputs, trace=False)
    return out


# revision 27
# speedup vs baseline: 1.2329x; 1.2329x over previous
"""Trainium2 Bass kernel: GroupNorm + single-head self-attention block.

Reference computation (per batch b):
    xn = GroupNorm(x, 16 groups, eps=1e-5) * gamma + beta
    q/k/v = W @ xn + b          (1x1 conv == channel matmul), [C, N]
    S = (q^T k) / sqrt(C)       [N, N]
    A = softmax_j(S)
    O = v @ A^T                 [C, N]
    y = wo @ O + bo + x

Shapes: B=4, C=256, H=W=64 -> N=4096.

Sharding: 8 cores = 4 batches x 2 query-halves.  Each core receives the
full x[b] with its query half permuted to the front, computes xn / v
for all N keys (cheap, avoids any collectives) and runs attention for
its 2048 queries.  The device program is identical on all cores (SPMD).

Algebraic restructuring (host-side, exact):
  - S^T[j,i] = sum_c k[c,j] q[c,i] = xn^T WQK xn with WQK = wq^T wk
    folded on the host; the per-query bias term from bk shifts all
    scores of a query equally and is dropped (softmax-invariant), the
    bq term survives as bqk = wk^T bq.
  - wo is folded into v: out = wo (v A_n^T) = (WOV xn + wo bv) A_n^T
    with WOV = wo wv.  The attention-value matmul then directly
    produces the final projection.

Device algorithm (per core):
  - GroupNorm stats via bn_stats/bn_aggr per channel + PE matmul with a
    group-indicator matrix for the cross-partition (channel) reduction.
  - qk = WQK^T xn + bqk for the 2048 local queries.
  - Scores computed TRANSPOSED per key-tile: S^T = xn^T qk, so both
    operands are natural [C, *] layouts (no transposes anywhere).
  - exp without max-subtraction (scores ~ N(0,1); fp32 exp is safe).
  - softmax denominator: ones-vector matmul over partitions on PE,
    broadcast back via a 0-stride-partition DMA, reciprocal on DVE.
  - out = v'^T A^T accumulated in PSUM, then *recip + residual on DVE.

Big matmuls run in float32r (full-rate fp32 PE mode).  fp32r operands
must be produced "rounded" by a compute engine, so every matmul input
tile is written by DVE/ACT with a float32r output dtype.
"""

import sys

sys.path.insert(0, "/opt/trn_rl_repo")

from contextlib import ExitStack

import numpy as np

import concourse.bacc as bacc
import concourse.bass as bass
import concourse.mybir as mybir
import concourse.tile as tile

B, C, H, W = 4, 256, 64, 64
N = H * W              # keys per batch
GROUPS = 16
EPS = 1e-5
NCORES = 8
QSPLIT = NCORES // B   # query shards per batch
NQ = N // QSPLIT       # queries per core
P = 128
CCH = C // P           # channel chunks (2)
IB = 512               # query block (one PSUM bank of f32)
NIB = NQ // IB         # query blocks per core
NJT = N // P           # key tiles (32)
GSZ = C // GROUPS      # channels per group (16)

F32 = mybir.dt.float32
F32R = mybir.dt.float32r
AF = mybir.ActivationFunctionType
OP = mybir.AluOpType


def build_nc(mm_dtype: str = "f32r"):
    """Emit the single-core SPMD program."""
    fp8_dr = mm_dtype.endswith("+fp8")
    base = mm_dtype.replace("+fp8", "")
    DTM = {"f32r": F32R, "bf16": mybir.dt.bfloat16, "f32": F32}[base]
    FP8 = mybir.dt.float8e4
    DTV = FP8 if fp8_dr else DTM   # dtype of the at / v' operands
    nc = bacc.Bacc()

    x_d = nc.declare_dram_parameter("x", [C, N], F32, isOutput=False)
    wqk_d = nc.declare_dram_parameter("wqk", [C, C], F32, isOutput=False)
    wovT_d = nc.declare_dram_parameter("wovT", [C, C], F32, isOutput=False)
    gamma_d = nc.declare_dram_parameter("gamma", [C], F32, isOutput=False)
    beta_d = nc.declare_dram_parameter("beta", [C], F32, isOutput=False)
    bqk_d = nc.declare_dram_parameter("bqk", [C], F32, isOutput=False)
    bvp_d = nc.declare_dram_parameter("bvp", [C], F32, isOutput=False)
    bo_d = nc.declare_dram_parameter("bo", [C], F32, isOutput=False)
    gind_d = nc.declare_dram_parameter("gind", [CCH, P, GROUPS], F32, isOutput=False)
    gindT_d = nc.declare_dram_parameter("gindT", [CCH, GROUPS, P], F32, isOutput=False)
    y_d = nc.declare_dram_parameter("y", [C, NQ], F32, isOutput=True)

    with tile.TileContext(nc) as tc, ExitStack() as ctx:
        const = ctx.enter_context(tc.tile_pool(name="const", bufs=1))
        data = ctx.enter_context(tc.tile_pool(name="data", bufs=1))

        # ---- weights: DMA to f32 staging, DVE-copy to fp32r tiles ----
        stage = ctx.enter_context(tc.tile_pool(name="stage", bufs=1))

        # fp32r lhsT free-dim counts must be even -> ones "column" is [P, 2]
        # (memset cannot emit fp32r; stage in f32 and DVE-copy to round)
        ones_f = const.tile([P, P], F32, name="ones_f")
        nc.vector.memset(ones_f, 1.0)
        ones_col2 = const.tile([P, 2], DTM, name="ones_col2")
        nc.vector.tensor_copy(ones_col2, ones_f[:, 0:2])
        ones_row_r = const.tile([1, P], DTM, name="ones_row_r")
        nc.vector.tensor_copy(ones_row_r, ones_f[0:1, :])
        if fp8_dr:
            # DoubleRow ones "column": [K, 2 pair-slices, M=16] -- the pair
            # dim stride must be 16B-aligned, so M is padded to 16
            ones_dr = const.tile([P, 2, 16], FP8, name="ones_dr")
            nc.vector.tensor_copy(
                ones_dr, ones_f[:, 0:32].rearrange("p (a b) -> p a b", a=2)
            )
            neg_ln16 = const.tile([P, 1], F32, name="neg_ln16")
            nc.vector.memset(neg_ln16, -2.772588722239781)  # -ln(16)
        # PE HAM warm-up scaffolding: the clock gate only reaches 2.4 GHz
        # after ~3.4us of sustained activity and re-throttles after an idle
        # window, so burn dummy matmuls during the DMA/GroupNorm prologue
        # (PE is otherwise idle there) and drip data-dependent "pings" so
        # the gate never sees an idle window before the real matmuls start.
        warm_src_f = const.tile([P, 512], F32, name="warm_src_f")
        nc.vector.memset(warm_src_f, 0.0)
        warm_src = const.tile([P, 512], DTM, name="warm_src")
        nc.vector.tensor_copy(warm_src, warm_src_f)
        def load_w(handle, nm):
            tiles = []
            for ch in range(CCH):
                s = stage.tile([P, C], F32, name=f"{nm}{ch}_s", tag=f"{nm}{ch}_s")
                nc.sync.dma_start(out=s, in_=handle[ch * P:(ch + 1) * P, :])
                t = const.tile([P, C], DTM, name=f"{nm}{ch}")
                nc.vector.tensor_copy(t, s)
                tiles.append(t)
            return tiles

        wqk = load_w(wqk_d, "wqk")      # [c, c'] chunks; lhsT for qk proj
        wovT = load_w(wovT_d, "wovT")   # [c', o] chunks; rhs for v' proj

        def load_vec(handle, nm):
            tiles = []
            for ch in range(CCH):
                t = const.tile([P, 1], F32, name=f"{nm}{ch}")
                nc.sync.dma_start(
                    out=t, in_=handle[ch * P:(ch + 1) * P].unsqueeze(1)
                )
                tiles.append(t)
            return tiles

        gamma = load_vec(gamma_d, "gamma")
        beta = load_vec(beta_d, "beta")
        bqk = load_vec(bqk_d, "bqk")
        bo = load_vec(bo_d, "bo")

        bvp_s = stage.tile([1, C], F32, name="bvp_s")
        nc.sync.dma_start(out=bvp_s, in_=bvp_d[:].unsqueeze(0))
        bvp_row = const.tile([1, C], DTM, name="bvp_row")
        nc.vector.tensor_copy(bvp_row, bvp_s)

        gind = []
        gindT = []
        for ch in range(CCH):
            gi = const.tile([P, GROUPS], F32, name=f"gind{ch}")
            nc.sync.dma_start(out=gi, in_=gind_d[ch])
            gind.append(gi)
            gt = const.tile([GROUPS, P], F32, name=f"gindT{ch}")
            nc.sync.dma_start(out=gt, in_=gindT_d[ch])
            gindT.append(gt)


        # ---- x in (staging pool released after GroupNorm) ----
        xn = data.tile([P, CCH, N], DTM, name="xn")
        resid = data.tile([P, CCH, NQ], F32, name="resid")


        with tc.tile_pool(name="xf_pool", bufs=1) as xf_pool, \
             tc.tile_pool(name="gn_psum", bufs=1, space="PSUM") as gn_psum, \
             tc.tile_pool(name="warm_psum", bufs=1, space="PSUM") as warm_psum, \
             tc.tile_pool(name="gn_sb", bufs=1) as gn_sb:
            warm_ps = warm_psum.tile([P, 512], F32, name="warm_ps")

            def warm(rhs=None, n=1):
                # M=2 keeps the HAM activity monitor fed at ~1/64th of the
                # PE-array power (wide bursts trip the firmware throttle)
                for _ in range(n):
                    nc.tensor.matmul(
                        warm_ps[:2, :512] if rhs is None else warm_ps[:2, :rhs.shape[-1]],
                        lhsT=ones_col2 if rhs is None else ones_f[:, 0:2],
                        rhs=warm_src if rhs is None else rhs,
                        start=True, stop=True, skip_group_check=True,
                    )

            warm(n=26)  # ~3.5us+ dense burst at t=0 -> gate opens early
            xf = xf_pool.tile([P, CCH, N], F32, name="xf")
            NS = N // 512  # bn_stats subgroups; DMA per subgroup to overlap
            for ch in range(CCH):
                for sg in range(NS):
                    nc.sync.dma_start(
                        out=xf[:, ch, sg * 512:(sg + 1) * 512],
                        in_=x_d[ch * P:(ch + 1) * P, sg * 512:(sg + 1) * 512],
                    )
            # ---- GroupNorm stats ----
            pc = []  # per-channel [mean, mean^2 + var] per chunk
            for ch in range(CCH):
                st6 = gn_sb.tile([P, NS, 6], F32, name=f"st6_{ch}")
                for sg in range(NS):
                    nc.vector.bn_stats(
                        out=st6[:, sg, :], in_=xf[:, ch, sg * 512:(sg + 1) * 512]
                    )
                    warm(rhs=st6[:, sg, :])
                mv = gn_sb.tile([P, 2], F32, name=f"mv{ch}")
                nc.vector.bn_aggr(out=mv, in_=st6)
                pcs = gn_sb.tile([P, 2], F32, name=f"pcs{ch}")
                nc.vector.tensor_copy(pcs[:, 0:1], mv[:, 0:1])
                # pcs[:,1] = mean^2 + var  (-> group E[x^2] after averaging)
                msq = gn_sb.tile([P, 1], F32, name=f"msq{ch}")
                nc.vector.tensor_mul(msq, mv[:, 0:1], mv[:, 0:1])
                nc.vector.tensor_add(pcs[:, 1:2], mv[:, 1:2], msq)
                pc.append(pcs)

            # residual (+ bo) for the local query half (ACT is idle here)
            for ch in range(CCH):
                nc.scalar.activation(
                    out=resid[:, ch, :], in_=xf[:, ch, :NQ], func=AF.Identity,
                    bias=bo[ch], scale=1.0,
                )

            gs_ps = gn_psum.tile([GROUPS, 2], F32, name="gs_ps")
            for ch in range(CCH):
                nc.tensor.matmul(
                    gs_ps, lhsT=gind[ch], rhs=pc[ch],
                    start=(ch == 0), stop=(ch == CCH - 1),
                )
            # per-channel stats are already means -> average over the GSZ
            # channels of each group
            gs = gn_sb.tile([GROUPS, 2], F32, name="gs")
            nc.scalar.mul(gs, gs_ps, 1.0 / GSZ)
            gvar = gn_sb.tile([GROUPS, 1], F32, name="gvar")
            gmsq = gn_sb.tile([GROUPS, 1], F32, name="gmsq")
            nc.vector.tensor_mul(gmsq, gs[:, 0:1], gs[:, 0:1])
            nc.vector.tensor_sub(gvar, gs[:, 1:2], gmsq)
            # rstd = 1/sqrt(var+eps)
            gstd = gn_sb.tile([GROUPS, 1], F32, name="gstd")
            eps_t = gn_sb.tile([GROUPS, 1], F32, name="eps_t")
            nc.vector.memset(eps_t, EPS)
            nc.scalar.activation(
                out=gstd, in_=gvar, func=AF.Sqrt, bias=eps_t, scale=1.0
            )
            gmr = gn_sb.tile([GROUPS, 2], F32, name="gmr")
            nc.vector.tensor_copy(gmr[:, 0:1], gs[:, 0:1])
            nc.vector.reciprocal(gmr[:, 1:2], gstd)

            # broadcast group (mean, rstd) back to channels, build affine
            for ch in range(CCH):
                cb_ps = gn_psum.tile([P, 2], F32, name="cb_ps", tag="cb_ps")
                nc.tensor.matmul(cb_ps, lhsT=gindT[ch], rhs=gmr,
                                 start=True, stop=True)
                cb = gn_sb.tile([P, 2], F32, name=f"cb{ch}")
                nc.vector.tensor_copy(cb, cb_ps)
                scale = gn_sb.tile([P, 1], F32, name=f"scale{ch}")
                nc.vector.tensor_mul(scale, gamma[ch], cb[:, 1:2])
                shift = gn_sb.tile([P, 1], F32, name=f"shift{ch}")
                nc.vector.tensor_mul(shift, cb[:, 0:1], scale)
                nc.vector.tensor_sub(shift, beta[ch], shift)
                # xn = x * scale + shift (column blocks -> projections
                # on early columns can start while later ones convert)
                for xb in range(4):
                    xsl = slice(xb * (N // 4), (xb + 1) * (N // 4))
                    nc.vector.tensor_scalar(
                        out=xn[:, ch, xsl], in0=xf[:, ch, xsl],
                        scalar1=scale, scalar2=shift, op0=OP.mult, op1=OP.add,
                    )
                warm(rhs=cb)

        # ---- projections ----
        qk = data.tile([P, CCH, NQ], DTM, name="qk")    # WQK^T xn + bqk
        vT = data.tile([P, NJT, C], DTV, name="vT")     # (WOV xn)^T + wo bv

        with tc.tile_pool(name="pj_psum", bufs=3, space="PSUM") as pj_psum:
            # v'-bias row broadcast once: b_sb[j, o] = bvp[o]
            bps = pj_psum.tile([P, C], F32, name="bps", tag="vT_ps")
            nc.tensor.matmul(bps, lhsT=ones_row_r, rhs=bvp_row,
                             start=True, stop=True)
            b_sb = const.tile([P, C], F32, name="b_sb")
            nc.vector.tensor_copy(b_sb, bps)
            # qk[c', i] = sum_c WQK[c, c'] xn[c, i] + bqk[c']
            for oc in range(CCH):
                for it in range(NQ // 512):
                    ps = pj_psum.tile([P, 512], F32, name="qk_ps", tag="qk_ps")
                    for ch in range(CCH):
                        nc.tensor.matmul(
                            ps,
                            lhsT=wqk[ch][:, oc * P:(oc + 1) * P],
                            rhs=xn[:, ch, it * 512:(it + 1) * 512],
                            start=(ch == 0), stop=(ch == CCH - 1),
                        )
                    nc.vector.tensor_scalar_add(
                        qk[:, oc, it * 512:(it + 1) * 512], ps, scalar1=bqk[oc]
                    )
            # vT[j, o] = sum_c' xn[c', j] WOV[o, c'] + (wo bv)[o]
            for jt in range(NJT):
                ps = pj_psum.tile([P, C], F32, name="vT_ps", tag="vT_ps")
                for ch in range(CCH):
                    nc.tensor.matmul(
                        ps,
                        lhsT=xn[:, ch, jt * P:(jt + 1) * P],
                        rhs=wovT[ch],
                        start=(ch == 0), stop=(ch == CCH - 1),
                    )
                nc.vector.tensor_add(vT[:, jt, :], ps, b_sb)

        # ---- attention ----
        with tc.tile_pool(name="st_psum", bufs=2, space="PSUM") as st_psum, \
             tc.tile_pool(name="o_psum", bufs=1, space="PSUM") as o_psum, \
             tc.tile_pool(name="sm_psum", bufs=1, space="PSUM") as sm_psum, \
             tc.tile_pool(name="at_pool", bufs=6) as at_pool, \
             tc.tile_pool(name="fin", bufs=2) as fin:
            for ib in range(NIB):
                isl = slice(ib * IB, (ib + 1) * IB)
                sums_ps = sm_psum.tile(
                    [16 if fp8_dr else 2, IB], F32, name="sums_ps", tag="sums"
                )
                o_ps = [
                    o_psum.tile([P, IB], F32, name=f"o_ps{cc}", tag=f"o{cc}")
                    for cc in range(CCH)
                ]
                # Software-pipelined on key-tile PAIRS: the score PSUM
                # tile holds two key-tiles (2 banks) so ONE exp covers the
                # pair and writes the fp8 DoubleRow [K, 2, N] layout
                # directly.  DR matmuls consume the pair with a 1-pair lag
                # so their waits are pre-satisfied.
                if fp8_dr:
                    PLAG = 1
                    npair = NJT // 2
                    ats = {}
                    for p in range(npair + PLAG):
                        if p < npair:
                            stp = st_psum.tile([P, 2, IB], F32, name="stp", tag="st")
                            for m in range(2):
                                jt = 2 * p + m
                                jsl = slice(jt * P, (jt + 1) * P)
                                for ch in range(CCH):
                                    nc.tensor.matmul(
                                        stp[:, m, :],
                                        lhsT=xn[:, ch, jsl],
                                        rhs=qk[:, ch, isl],
                                        start=(ch == 0), stop=(ch == CCH - 1),
                                    )
                            atp = at_pool.tile([P, 2, IB], FP8, name="atp", tag="at")
                            # A^T = exp(S^T/16 - ln 16); the -ln16 keeps fp8e4
                            # in range and cancels in the normalization
                            nc.scalar.activation(
                                out=atp.rearrange("p a b -> p (a b)"),
                                in_=stp.rearrange("p a b -> p (a b)"),
                                func=AF.Exp, scale=1.0 / 16.0, bias=neg_ln16,
                            )
                            ats[p] = atp
                        if p >= PLAG:
                            pg = p - PLAG
                            atp = ats.pop(pg)
                            nc.tensor.matmul(
                                sums_ps, lhsT=ones_dr, rhs=atp,
                                start=(pg == 0), stop=(pg == npair - 1),
                                perf_mode=mybir.MatmulPerfMode.DoubleRow,
                            )
                            for cc in range(CCH):
                                nc.tensor.matmul(
                                    o_ps[cc],
                                    lhsT=vT[:, 2 * pg:2 * pg + 2,
                                            cc * P:(cc + 1) * P],
                                    rhs=atp,
                                    start=(pg == 0), stop=(pg == npair - 1),
                                    perf_mode=mybir.MatmulPerfMode.DoubleRow,
                                )
                else:
                    LAG = 2
                    ats = {}
                    for jt in range(NJT + LAG):
                        if jt < NJT:
                            jsl = slice(jt * P, (jt + 1) * P)
                            st = st_psum.tile([P, IB], F32, name="st", tag="st")
                            for ch in range(CCH):
                                nc.tensor.matmul(
                                    st,
                                    lhsT=xn[:, ch, jsl],
                                    rhs=qk[:, ch, isl],
                                    start=(ch == 0), stop=(ch == CCH - 1),
                                )
                            at = at_pool.tile([P, IB], DTM, name="at", tag="at")
                            nc.scalar.activation(
                                out=at, in_=st, func=AF.Exp, scale=1.0 / 16.0
                            )
                            ats[jt] = at
                        if jt >= LAG and (jt - LAG) % 2 == 1:
                            for g in (jt - LAG - 1, jt - LAG):
                                at_g = ats.pop(g)
                                nc.tensor.matmul(
                                    sums_ps, lhsT=ones_col2, rhs=at_g,
                                    start=(g == 0), stop=(g == NJT - 1),
                                )
                                for cc in range(CCH):
                                    nc.tensor.matmul(
                                        o_ps[cc],
                                        lhsT=vT[:, g, cc * P:(cc + 1) * P],
                                        rhs=at_g,
                                        start=(g == 0), stop=(g == NJT - 1),
                                    )

                # free the accumulators quickly so the next block's PE
                # matmuls don't wait on the normalization chain
                o_sb = []
                for cc in range(CCH):
                    t = fin.tile([P, IB], F32, name=f"o_sb{cc}", tag=f"osb{cc}")
                    nc.vector.tensor_copy(t, o_ps[cc])
                    o_sb.append(t)

                # denominator -> [128, IB] broadcast (PE) + reciprocal (DVE)
                sums_row = fin.tile([1, IB], F32, name="sums_row", tag="sums_row")
                nc.vector.tensor_copy(sums_row, sums_ps[0:1, :])
                rb_ps = sm_psum.tile([P, IB], F32, name="rb_ps", tag="rb")
                nc.tensor.matmul(rb_ps, lhsT=ones_f[0:1, :], rhs=sums_row,
                                 start=True, stop=True)
                rb = fin.tile([P, IB], F32, name="rb", tag="rbs")
                nc.vector.reciprocal(rb, rb_ps)

                for oc in range(CCH):
                    t = fin.tile([P, IB], F32, name="t_sb", tag="t_sb")
                    nc.vector.tensor_mul(t, o_sb[oc], rb)
                    out_sb = fin.tile([P, IB], F32, name="out_sb", tag="out_sb")
                    nc.vector.tensor_add(out_sb, t, resid[:, oc, isl])
                    nc.sync.dma_start(
                        out=y_d[oc * P:(oc + 1) * P, isl], in_=out_sb
                    )
    nc.finalize()
    return nc


_NC_CACHE = {}


def _get_nc(mm_dtype="f32r"):
    if mm_dtype not in _NC_CACHE:
        _NC_CACHE[mm_dtype] = build_nc(mm_dtype)
    return _NC_CACHE[mm_dtype]


def make_in_maps(inputs):
    """Shard full inputs into per-core input maps (host-side weight folding)."""
    x = np.asarray(inputs["x"], np.float32).reshape(B, C, N)
    gamma = np.asarray(inputs["gamma"], np.float32)
    beta = np.asarray(inputs["beta"], np.float32)
    wq = np.asarray(inputs["wq"], np.float64)
    bq = np.asarray(inputs["bq"], np.float64)
    wk = np.asarray(inputs["wk"], np.float64)
    wv = np.asarray(inputs["wv"], np.float64)
    bv = np.asarray(inputs["bv"], np.float64)
    wo = np.asarray(inputs["wo"], np.float64)
    bo = np.asarray(inputs["bo"], np.float32)

    # S^T = xn^T (wq^T wk) xn + (wk^T bq) broadcast over keys
    wqk = np.ascontiguousarray((wq.T @ wk).astype(np.float32))      # [c, c']
    bqk = (wk.T @ bq).astype(np.float32)                            # [c']
    # out = (wo wv xn + wo bv) A_n^T
    wovT = np.ascontiguousarray((wo @ wv).T.astype(np.float32))     # [c', o]
    bvp = (wo @ bv).astype(np.float32)                              # [o]

    gind = np.zeros((CCH, P, GROUPS), np.float32)
    for ch in range(CCH):
        for p in range(P):
            gind[ch, p, (ch * P + p) // GSZ] = 1.0
    gindT = np.ascontiguousarray(gind.transpose(0, 2, 1))

    shared = {
        "wqk": wqk, "wovT": wovT,
        "gamma": gamma, "beta": beta,
        "bqk": bqk, "bvp": bvp, "bo": bo,
        "gind": gind, "gindT": gindT,
    }
    in_maps = []
    for core in range(NCORES):
        b, h = divmod(core, QSPLIT)
        if h == 0:
            xc = x[b]
        else:
            xc = np.concatenate(
                [x[b][:, h * NQ:(h + 1) * NQ], x[b][:, :h * NQ],
                 x[b][:, (h + 1) * NQ:]], axis=1,
            )
        in_maps.append({"x": np.ascontiguousarray(xc), **shared})
    return in_maps


def gather_output(results):
    y = np.empty((B, C, N), np.float32)
    for core in range(NCORES):
        b, h = divmod(core, QSPLIT)
        y[b][:, h * NQ:(h + 1) * NQ] = results[core]["y"]
    return y.reshape(B, C, H, W)


def _run_traced(nc, in_maps, core_ids, tmpdir=None):
    """Replicates run_bass_kernel_spmd's axon trace branch; this image
    lacks antenv.axon_hooks, so drive the NTFF hook via ctypes directly."""
    import glob
    import tempfile

    import gauge.profiler
    from concourse import bass2jax
    from concourse._compat import FishPath
    from concourse.bass_utils import BassKernelResults, _process_ntff_profile
    from trn_agent_boot.trn_boot import _ntff_profile_via_ctypes

    hook = _ntff_profile_via_ctypes("/opt/axon/libaxon_pjrt.so")
    if tmpdir is None:
        tmpdir = tempfile.mkdtemp(prefix="bassprof_")
    if hook is None:
        results = bass2jax.run_bass_via_pjrt(nc, in_maps, n_cores=len(core_ids))
        return BassKernelResults(results, None, None, None)
    with hook(tmpdir, [0]):
        results = bass2jax.run_bass_via_pjrt(nc, in_maps, n_cores=len(core_ids))
    if not glob.glob(f"{tmpdir}/*_body*.ntff"):
        print(f"no NTFF produced in {tmpdir}")
        return BassKernelResults(results, None, None, None)
    profile = gauge.profiler.Profile(
        profile_path=FishPath(tmpdir),
        kernel_dev_mode=True,
        profile_on_exit=False,
        bass_kernel=nc.m,
        offline_processing=True,
        fname="*_body*",
        metadata={},
    )
    return _process_ntff_profile(
        profile, tmpdir, nc, core_ids, None, False, {}, False
    ).as_bass_kernel_results(results)


def run_spmd(inputs, trace=False, mm_dtype="f32r", tmpdir=None):
    from concourse.bass_utils import run_bass_kernel_spmd

    nc = _get_nc(mm_dtype)
    in_maps = make_in_maps(inputs)
    if trace:
        res = _run_traced(nc, in_maps, list(range(NCORES)), tmpdir=tmpdir)
    else:
        res = run_bass_kernel_spmd(nc, in_maps, list(range(NCORES)), trace=False)
    return gather_output(res.results), res


def kernel(**inputs) -> np.ndarray:
    out, _ = run_spmd(inputs, trace=False)
    return out


# revision 29
# speedup vs baseline: 1.2529x; 1.0162x over previous
"""Trainium2 Bass kernel: GroupNorm + single-head self-attention block.

Reference computation (per batch b):
    xn = GroupNorm(x, 16 groups, eps=1e-5) * gamma + beta
    q/k/v = W @ xn + b          (1x1 conv == channel matmul), [C, N]
    S = (q^T k) / sqrt(C)       [N, N]
    A = softmax_j(S)
    O = v @ A^T                 [C, N]
    y = wo @ O + bo + x

Shapes: B=4, C=256, H=W=64 -> N=4096.

Sharding: 8 cores = 4 batches x 2 query-halves.  Each core receives the
full x[b] with its query half permuted to the front, computes xn / v
for all N keys (cheap, avoids any collectives) and runs attention for
its 2048 queries.  The device program is identical on all cores (SPMD).

Algebraic restructuring (host-side, exact):
  - S^T[j,i] = sum_c k[c,j] q[c,i] = xn^T WQK xn with WQK = wq^T wk
    folded on the host; the per-query bias term from bk shifts all
    scores of a query equally and is dropped (softmax-invariant), the
    bq term survives as bqk = wk^T bq.
  - wo is folded into v: out = wo (v A_n^T) = (WOV xn + wo bv) A_n^T
    with WOV = wo wv.  The attention-value matmul then directly
    produces the final projection.

Device algorithm (per core):
  - GroupNorm stats via bn_stats/bn_aggr per channel + PE matmul with a
    group-indicator matrix for the cross-partition (channel) reduction.
  - qk = WQK^T xn + bqk for the 2048 local queries.
  - Scores computed TRANSPOSED per key-tile: S^T = xn^T qk, so both
    operands are natural [C, *] layouts (no transposes anywhere).
  - exp without max-subtraction (scores ~ N(0,1); fp32 exp is safe).
  - softmax denominator: ones-vector matmul over partitions on PE,
    broadcast back via a 0-stride-partition DMA, reciprocal on DVE.
  - out = v'^T A^T accumulated in PSUM, then *recip + residual on DVE.

Big matmuls run in float32r (full-rate fp32 PE mode).  fp32r operands
must be produced "rounded" by a compute engine, so every matmul input
tile is written by DVE/ACT with a float32r output dtype.
"""

import sys

sys.path.insert(0, "/opt/trn_rl_repo")

from contextlib import ExitStack

import numpy as np

import concourse.bacc as bacc
import concourse.bass as bass
import concourse.mybir as mybir
import concourse.tile as tile

B, C, H, W = 4, 256, 64, 64
N = H * W              # keys per batch
GROUPS = 16
EPS = 1e-5
NCORES = 8
QSPLIT = NCORES // B   # query shards per batch
NQ = N // QSPLIT       # queries per core
P = 128
CCH = C // P           # channel chunks (2)
IB = 512               # query block (one PSUM bank of f32)
NIB = NQ // IB         # query blocks per core
NJT = N // P           # key tiles (32)
GSZ = C // GROUPS      # channels per group (16)

F32 = mybir.dt.float32
F32R = mybir.dt.float32r
AF = mybir.ActivationFunctionType
OP = mybir.AluOpType


def build_nc(mm_dtype: str = "f32r"):
    """Emit the single-core SPMD program."""
    fp8_dr = mm_dtype.endswith("+fp8")
    base = mm_dtype.replace("+fp8", "")
    DTM = {"f32r": F32R, "bf16": mybir.dt.bfloat16, "f32": F32}[base]
    FP8 = mybir.dt.float8e4
    DTV = FP8 if fp8_dr else DTM   # dtype of the at / v' operands
    nc = bacc.Bacc()

    x_d = nc.declare_dram_parameter("x", [C, N], F32, isOutput=False)
    wqk_d = nc.declare_dram_parameter("wqk", [C, C], F32, isOutput=False)
    wovT_d = nc.declare_dram_parameter("wovT", [C, C], F32, isOutput=False)
    gamma_d = nc.declare_dram_parameter("gamma", [C], F32, isOutput=False)
    beta_d = nc.declare_dram_parameter("beta", [C], F32, isOutput=False)
    bqk_d = nc.declare_dram_parameter("bqk", [C], F32, isOutput=False)
    bvp_d = nc.declare_dram_parameter("bvp", [C], F32, isOutput=False)
    bo_d = nc.declare_dram_parameter("bo", [C], F32, isOutput=False)
    gind_d = nc.declare_dram_parameter("gind", [CCH, P, GROUPS], F32, isOutput=False)
    gindT_d = nc.declare_dram_parameter("gindT", [CCH, GROUPS, P], F32, isOutput=False)
    y_d = nc.declare_dram_parameter("y", [C, NQ], F32, isOutput=True)

    with tile.TileContext(nc) as tc, ExitStack() as ctx:
        const = ctx.enter_context(tc.tile_pool(name="const", bufs=1))
        data = ctx.enter_context(tc.tile_pool(name="data", bufs=1))

        # ---- weights: DMA to f32 staging, DVE-copy to fp32r tiles ----
        stage = ctx.enter_context(tc.tile_pool(name="stage", bufs=1))

        # fp32r lhsT free-dim counts must be even -> ones "column" is [P, 2]
        # (memset cannot emit fp32r; stage in f32 and DVE-copy to round)
        ones_f = const.tile([P, P], F32, name="ones_f")
        nc.vector.memset(ones_f, 1.0)
        ones_col2 = const.tile([P, 2], DTM, name="ones_col2")
        nc.vector.tensor_copy(ones_col2, ones_f[:, 0:2])
        ones_row_r = const.tile([1, P], DTM, name="ones_row_r")
        nc.vector.tensor_copy(ones_row_r, ones_f[0:1, :])
        if fp8_dr:
            # DoubleRow ones "column": [K, 2 pair-slices, M=16] -- the pair
            # dim stride must be 16B-aligned, so M is padded to 16
            ones_dr = const.tile([P, 2, 16], FP8, name="ones_dr")
            nc.vector.tensor_copy(
                ones_dr, ones_f[:, 0:32].rearrange("p (a b) -> p a b", a=2)
            )
            neg_ln16 = const.tile([P, 1], F32, name="neg_ln16")
            nc.vector.memset(neg_ln16, -2.772588722239781)  # -ln(16)
        # PE HAM warm-up scaffolding: the clock gate only reaches 2.4 GHz
        # after ~3.4us of sustained activity and re-throttles after an idle
        # window, so burn dummy matmuls during the DMA/GroupNorm prologue
        # (PE is otherwise idle there) and drip data-dependent "pings" so
        # the gate never sees an idle window before the real matmuls start.
        warm_src_f = const.tile([P, 512], F32, name="warm_src_f")
        nc.vector.memset(warm_src_f, 0.0)
        warm_src = const.tile([P, 512], DTM, name="warm_src")
        nc.vector.tensor_copy(warm_src, warm_src_f)
        def load_w(handle, nm):
            tiles = []
            for ch in range(CCH):
                s = stage.tile([P, C], F32, name=f"{nm}{ch}_s", tag=f"{nm}{ch}_s")
                nc.scalar.dma_start(out=s, in_=handle[ch * P:(ch + 1) * P, :])
                t = const.tile([P, C], DTM, name=f"{nm}{ch}")
                nc.vector.tensor_copy(t, s)
                tiles.append(t)
            return tiles

        wqk = load_w(wqk_d, "wqk")      # [c, c'] chunks; lhsT for qk proj
        wovT = load_w(wovT_d, "wovT")   # [c', o] chunks; rhs for v' proj

        def load_vec(handle, nm):
            tiles = []
            for ch in range(CCH):
                t = const.tile([P, 1], F32, name=f"{nm}{ch}")
                nc.scalar.dma_start(
                    out=t, in_=handle[ch * P:(ch + 1) * P].unsqueeze(1)
                )
                tiles.append(t)
            return tiles

        gamma = load_vec(gamma_d, "gamma")
        beta = load_vec(beta_d, "beta")
        bqk = load_vec(bqk_d, "bqk")
        bo = load_vec(bo_d, "bo")

        bvp_s = stage.tile([1, C], F32, name="bvp_s")
        nc.scalar.dma_start(out=bvp_s, in_=bvp_d[:].unsqueeze(0))
        bvp_row = const.tile([1, C], DTM, name="bvp_row")
        nc.vector.tensor_copy(bvp_row, bvp_s)

        gind = []
        gindT = []
        for ch in range(CCH):
            gi = const.tile([P, GROUPS], F32, name=f"gind{ch}")
            nc.scalar.dma_start(out=gi, in_=gind_d[ch])
            gind.append(gi)
            gt = const.tile([GROUPS, P], F32, name=f"gindT{ch}")
            nc.scalar.dma_start(out=gt, in_=gindT_d[ch])
            gindT.append(gt)


        # ---- x in (staging pool released after GroupNorm) ----
        xn = data.tile([P, CCH, N], DTM, name="xn")
        resid = data.tile([P, CCH, NQ], F32, name="resid")


        with tc.tile_pool(name="xf_pool", bufs=1) as xf_pool, \
             tc.tile_pool(name="gn_psum", bufs=1, space="PSUM") as gn_psum, \
             tc.tile_pool(name="warm_psum", bufs=1, space="PSUM") as warm_psum, \
             tc.tile_pool(name="gn_sb", bufs=1) as gn_sb:
            warm_ps = warm_psum.tile([P, 512], F32, name="warm_ps")

            def warm(rhs=None, n=1):
                # M=2 keeps the HAM activity monitor fed at ~1/64th of the
                # PE-array power (wide bursts trip the firmware throttle)
                for _ in range(n):
                    nc.tensor.matmul(
                        warm_ps[:2, :512] if rhs is None else warm_ps[:2, :rhs.shape[-1]],
                        lhsT=ones_col2 if rhs is None else ones_f[:, 0:2],
                        rhs=warm_src if rhs is None else rhs,
                        start=True, stop=True, skip_group_check=True,
                    )

            warm(n=26)  # ~3.5us+ dense burst at t=0 -> gate opens early
            xf = xf_pool.tile([P, CCH, N], F32, name="xf")
            NS = N // 512  # bn_stats subgroups; DMA per subgroup to overlap
            for ch in range(CCH):
                for sg in range(NS):
                    eng = nc.sync if (ch * NS + sg) % 2 == 0 else nc.gpsimd
                    eng.dma_start(
                        out=xf[:, ch, sg * 512:(sg + 1) * 512],
                        in_=x_d[ch * P:(ch + 1) * P, sg * 512:(sg + 1) * 512],
                    )
            # ---- GroupNorm stats ----
            pc = []  # per-channel [mean, mean^2 + var] per chunk
            for ch in range(CCH):
                st6 = gn_sb.tile([P, NS, 6], F32, name=f"st6_{ch}")
                for sg in range(NS):
                    nc.vector.bn_stats(
                        out=st6[:, sg, :], in_=xf[:, ch, sg * 512:(sg + 1) * 512]
                    )
                    warm(rhs=st6[:, sg, :])
                mv = gn_sb.tile([P, 2], F32, name=f"mv{ch}")
                nc.vector.bn_aggr(out=mv, in_=st6)
                pcs = gn_sb.tile([P, 2], F32, name=f"pcs{ch}")
                nc.vector.tensor_copy(pcs[:, 0:1], mv[:, 0:1])
                # pcs[:,1] = mean^2 + var  (-> group E[x^2] after averaging)
                msq = gn_sb.tile([P, 1], F32, name=f"msq{ch}")
                nc.vector.tensor_mul(msq, mv[:, 0:1], mv[:, 0:1])
                nc.vector.tensor_add(pcs[:, 1:2], mv[:, 1:2], msq)
                pc.append(pcs)

            # residual (+ bo) for the local query half (ACT is idle here)
            for ch in range(CCH):
                nc.scalar.activation(
                    out=resid[:, ch, :], in_=xf[:, ch, :NQ], func=AF.Identity,
                    bias=bo[ch], scale=1.0,
                )

            gs_ps = gn_psum.tile([GROUPS, 2], F32, name="gs_ps")
            for ch in range(CCH):
                nc.tensor.matmul(
                    gs_ps, lhsT=gind[ch], rhs=pc[ch],
                    start=(ch == 0), stop=(ch == CCH - 1),
                )
            # per-channel stats are already means -> average over the GSZ
            # channels of each group
            gs = gn_sb.tile([GROUPS, 2], F32, name="gs")
            nc.scalar.mul(gs, gs_ps, 1.0 / GSZ)
            gvar = gn_sb.tile([GROUPS, 1], F32, name="gvar")
            gmsq = gn_sb.tile([GROUPS, 1], F32, name="gmsq")
            nc.vector.tensor_mul(gmsq, gs[:, 0:1], gs[:, 0:1])
            nc.vector.tensor_sub(gvar, gs[:, 1:2], gmsq)
            # rstd = 1/sqrt(var+eps)
            gstd = gn_sb.tile([GROUPS, 1], F32, name="gstd")
            eps_t = gn_sb.tile([GROUPS, 1], F32, name="eps_t")
            nc.vector.memset(eps_t, EPS)
            nc.scalar.activation(
                out=gstd, in_=gvar, func=AF.Sqrt, bias=eps_t, scale=1.0
            )
            gmr = gn_sb.tile([GROUPS, 2], F32, name="gmr")
            nc.vector.tensor_copy(gmr[:, 0:1], gs[:, 0:1])
            nc.vector.reciprocal(gmr[:, 1:2], gstd)

            # broadcast group (mean, rstd) back to channels, build affine
            for ch in range(CCH):
                cb_ps = gn_psum.tile([P, 2], F32, name="cb_ps", tag="cb_ps")
                nc.tensor.matmul(cb_ps, lhsT=gindT[ch], rhs=gmr,
                                 start=True, stop=True)
                cb = gn_sb.tile([P, 2], F32, name=f"cb{ch}")
                nc.vector.tensor_copy(cb, cb_ps)
                scale = gn_sb.tile([P, 1], F32, name=f"scale{ch}")
                nc.vector.tensor_mul(scale, gamma[ch], cb[:, 1:2])
                shift = gn_sb.tile([P, 1], F32, name=f"shift{ch}")
                nc.vector.tensor_mul(shift, cb[:, 0:1], scale)
                nc.vector.tensor_sub(shift, beta[ch], shift)
                # xn = x * scale + shift (column blocks -> projections
                # on early columns can start while later ones convert)
                for xb in range(4):
                    xsl = slice(xb * (N // 4), (xb + 1) * (N // 4))
                    nc.vector.tensor_scalar(
                        out=xn[:, ch, xsl], in0=xf[:, ch, xsl],
                        scalar1=scale, scalar2=shift, op0=OP.mult, op1=OP.add,
                    )
                warm(rhs=cb)

        # ---- projections ----
        qk = data.tile([P, CCH, NQ], DTM, name="qk")    # WQK^T xn + bqk
        vT = data.tile([P, NJT, C], DTV, name="vT")     # (WOV xn)^T + wo bv

        with tc.tile_pool(name="pj_psum", bufs=3, space="PSUM") as pj_psum:
            # v'-bias row broadcast once: b_sb[j, o] = bvp[o]
            bps = pj_psum.tile([P, C], F32, name="bps", tag="vT_ps")
            nc.tensor.matmul(bps, lhsT=ones_row_r, rhs=bvp_row,
                             start=True, stop=True)
            b_sb = const.tile([P, C], F32, name="b_sb")
            nc.vector.tensor_copy(b_sb, bps)
            # qk[c', i] = sum_c WQK[c, c'] xn[c, i] + bqk[c']
            for oc in range(CCH):
                for it in range(NQ // 512):
                    ps = pj_psum.tile([P, 512], F32, name="qk_ps", tag="qk_ps")
                    for ch in range(CCH):
                        nc.tensor.matmul(
                            ps,
                            lhsT=wqk[ch][:, oc * P:(oc + 1) * P],
                            rhs=xn[:, ch, it * 512:(it + 1) * 512],
                            start=(ch == 0), stop=(ch == CCH - 1),
                        )
                    nc.vector.tensor_scalar_add(
                        qk[:, oc, it * 512:(it + 1) * 512], ps, scalar1=bqk[oc]
                    )
            # vT[j, o] = sum_c' xn[c', j] WOV[o, c'] + (wo bv)[o]
            for jt in range(NJT):
                ps = pj_psum.tile([P, C], F32, name="vT_ps", tag="vT_ps")
                for ch in range(CCH):
                    nc.tensor.matmul(
                        ps,
                        lhsT=xn[:, ch, jt * P:(jt + 1) * P],
                        rhs=wovT[ch],
                        start=(ch == 0), stop=(ch == CCH - 1),
                    )
                nc.vector.tensor_add(vT[:, jt, :], ps, b_sb)

        # ---- attention ----
        with tc.tile_pool(name="st_psum", bufs=2, space="PSUM") as st_psum, \
             tc.tile_pool(name="o_psum", bufs=1, space="PSUM") as o_psum, \
             tc.tile_pool(name="sm_psum", bufs=1, space="PSUM") as sm_psum, \
             tc.tile_pool(name="at_pool", bufs=6) as at_pool, \
             tc.tile_pool(name="fin", bufs=2) as fin:
            for ib in range(NIB):
                isl = slice(ib * IB, (ib + 1) * IB)
                sums_ps = sm_psum.tile(
                    [16 if fp8_dr else 2, IB], F32, name="sums_ps", tag="sums"
                )
                o_ps = [
                    o_psum.tile([P, IB], F32, name=f"o_ps{cc}", tag=f"o{cc}")
                    for cc in range(CCH)
                ]
                # Software-pipelined on key-tile PAIRS: the score PSUM
                # tile holds two key-tiles (2 banks) so ONE exp covers the
                # pair and writes the fp8 DoubleRow [K, 2, N] layout
                # directly.  DR matmuls consume the pair with a 1-pair lag
                # so their waits are pre-satisfied.
                if fp8_dr:
                    PLAG = 1
                    npair = NJT // 2
                    ats = {}
                    for p in range(npair + PLAG):
                        if p < npair:
                            stp = st_psum.tile([P, 2, IB], F32, name="stp", tag="st")
                            for m in range(2):
                                jt = 2 * p + m
                                jsl = slice(jt * P, (jt + 1) * P)
                                for ch in range(CCH):
                                    nc.tensor.matmul(
                                        stp[:, m, :],
                                        lhsT=xn[:, ch, jsl],
                                        rhs=qk[:, ch, isl],
                                        start=(ch == 0), stop=(ch == CCH - 1),
                                    )
                            atp = at_pool.tile([P, 2, IB], FP8, name="atp", tag="at")
                            # A^T = exp(S^T/16 - ln 16); the -ln16 keeps fp8e4
                            # in range and cancels in the normalization
                            nc.scalar.activation(
                                out=atp.rearrange("p a b -> p (a b)"),
                                in_=stp.rearrange("p a b -> p (a b)"),
                                func=AF.Exp, scale=1.0 / 16.0, bias=neg_ln16,
                            )
                            ats[p] = atp
                        if p >= PLAG:
                            pg = p - PLAG
                            atp = ats.pop(pg)
                            nc.tensor.matmul(
                                sums_ps, lhsT=ones_dr, rhs=atp,
                                start=(pg == 0), stop=(pg == npair - 1),
                                perf_mode=mybir.MatmulPerfMode.DoubleRow,
                            )
                            for cc in range(CCH):
                                nc.tensor.matmul(
                                    o_ps[cc],
                                    lhsT=vT[:, 2 * pg:2 * pg + 2,
                                            cc * P:(cc + 1) * P],
                                    rhs=atp,
                                    start=(pg == 0), stop=(pg == npair - 1),
                                    perf_mode=mybir.MatmulPerfMode.DoubleRow,
                                )
                else:
                    LAG = 2
                    ats = {}
                    for jt in range(NJT + LAG):
                        if jt < NJT:
                            jsl = slice(jt * P, (jt + 1) * P)
                            st = st_psum.tile([P, IB], F32, name="st", tag="st")
                            for ch in range(CCH):
                                nc.tensor.matmul(
                                    st,
                                    lhsT=xn[:, ch, jsl],
                                    rhs=qk[:, ch, isl],
                                    start=(ch == 0), stop=(ch == CCH - 1),
                                )
                            at = at_pool.tile([P, IB], DTM, name="at", tag="at")
                            nc.scalar.activation(
                                out=at, in_=st, func=AF.Exp, scale=1.0 / 16.0
                            )
                            ats[jt] = at
                        if jt >= LAG and (jt - LAG) % 2 == 1:
                            for g in (jt - LAG - 1, jt - LAG):
                                at_g = ats.pop(g)
                                nc.tensor.matmul(
                                    sums_ps, lhsT=ones_col2, rhs=at_g,
                                    start=(g == 0), stop=(g == NJT - 1),
                                )
                                for cc in range(CCH):
                                    nc.tensor.matmul(
                                        o_ps[cc],
                                        lhsT=vT[:, g, cc * P:(cc + 1) * P],
                                        rhs=at_g,
                                        start=(g == 0), stop=(g == NJT - 1),
                                    )

                # free the accumulators quickly so the next block's PE
                # matmuls don't wait on the normalization chain
                o_sb = []
                for cc in range(CCH):
                    t = fin.tile([P, IB], F32, name=f"o_sb{cc}", tag=f"osb{cc}")
                    nc.vector.tensor_copy(t, o_ps[cc])
                    o_sb.append(t)

                # denominator -> [128, IB] broadcast (PE) + reciprocal (DVE)
                sums_row = fin.tile([1, IB], F32, name="sums_row", tag="sums_row")
                nc.vector.tensor_copy(sums_row, sums_ps[0:1, :])
                rb_ps = sm_psum.tile([P, IB], F32, name="rb_ps", tag="rb")
                nc.tensor.matmul(rb_ps, lhsT=ones_f[0:1, :], rhs=sums_row,
                                 start=True, stop=True)
                rb = fin.tile([P, IB], F32, name="rb", tag="rbs")
                nc.vector.reciprocal(rb, rb_ps)

                for oc in range(CCH):
                    t = fin.tile([P, IB], F32, name="t_sb", tag="t_sb")
                    nc.vector.tensor_mul(t, o_sb[oc], rb)
                    out_sb = fin.tile([P, IB], F32, name="out_sb", tag="out_sb")
                    nc.vector.tensor_add(out_sb, t, resid[:, oc, isl])
                    nc.sync.dma_start(
                        out=y_d[oc * P:(oc + 1) * P, isl], in_=out_sb
                    )
    nc.finalize()
    return nc


_NC_CACHE = {}


def _get_nc(mm_dtype="f32r"):
    if mm_dtype not in _NC_CACHE:
        _NC_CACHE[mm_dtype] = build_nc(mm_dtype)
    return _NC_CACHE[mm_dtype]


def make_in_maps(inputs):
    """Shard full inputs into per-core input maps (host-side weight folding)."""
    x = np.asarray(inputs["x"], np.float32).reshape(B, C, N)
    gamma = np.asarray(inputs["gamma"], np.float32)
    beta = np.asarray(inputs["beta"], np.float32)
    wq = np.asarray(inputs["wq"], np.float64)
    bq = np.asarray(inputs["bq"], np.float64)
    wk = np.asarray(inputs["wk"], np.float64)
    wv = np.asarray(inputs["wv"], np.float64)
    bv = np.asarray(inputs["bv"], np.float64)
    wo = np.asarray(inputs["wo"], np.float64)
    bo = np.asarray(inputs["bo"], np.float32)

    # S^T = xn^T (wq^T wk) xn + (wk^T bq) broadcast over keys
    wqk = np.ascontiguousarray((wq.T @ wk).astype(np.float32))      # [c, c']
    bqk = (wk.T @ bq).astype(np.float32)                            # [c']
    # out = (wo wv xn + wo bv) A_n^T
    wovT = np.ascontiguousarray((wo @ wv).T.astype(np.float32))     # [c', o]
    bvp = (wo @ bv).astype(np.float32)                              # [o]

    gind = np.zeros((CCH, P, GROUPS), np.float32)
    for ch in range(CCH):
        for p in range(P):
            gind[ch, p, (ch * P + p) // GSZ] = 1.0
    gindT = np.ascontiguousarray(gind.transpose(0, 2, 1))

    shared = {
        "wqk": wqk, "wovT": wovT,
        "gamma": gamma, "beta": beta,
        "bqk": bqk, "bvp": bvp, "bo": bo,
        "gind": gind, "gindT": gindT,
    }
    in_maps = []
    for core in range(NCORES):
        b, h = divmod(core, QSPLIT)
        if h == 0:
            xc = x[b]
        else:
            xc = np.concatenate(
                [x[b][:, h * NQ:(h + 1) * NQ], x[b][:, :h * NQ],
                 x[b][:, (h + 1) * NQ:]], axis=1,
            )
        in_maps.append({"x": np.ascontiguousarray(xc), **shared})
    return in_maps


def gather_output(results):
    y = np.empty((B, C, N), np.float32)
    for core in range(NCORES):
        b, h = divmod(core, QSPLIT)
        y[b][:, h * NQ:(h + 1) * NQ] = results[core]["y"]
    return y.reshape(B, C, H, W)


def _run_traced(nc, in_maps, core_ids, tmpdir=None):
    """Replicates run_bass_kernel_spmd's axon trace branch; this image
    lacks antenv.axon_hooks, so drive the NTFF hook via ctypes directly."""
    import glob
    import tempfile

    import gauge.profiler
    from concourse import bass2jax
    from concourse._compat import FishPath
    from concourse.bass_utils import BassKernelResults, _process_ntff_profile
    from trn_agent_boot.trn_boot import _ntff_profile_via_ctypes

    hook = _ntff_profile_via_ctypes("/opt/axon/libaxon_pjrt.so")
    if tmpdir is None:
        tmpdir = tempfile.mkdtemp(prefix="bassprof_")
    if hook is None:
        results = bass2jax.run_bass_via_pjrt(nc, in_maps, n_cores=len(core_ids))
        return BassKernelResults(results, None, None, None)
    with hook(tmpdir, [0]):
        results = bass2jax.run_bass_via_pjrt(nc, in_maps, n_cores=len(core_ids))
    if not glob.glob(f"{tmpdir}/*_body*.ntff"):
        print(f"no NTFF produced in {tmpdir}")
        return BassKernelResults(results, None, None, None)
    profile = gauge.profiler.Profile(
        profile_path=FishPath(tmpdir),
        kernel_dev_mode=True,
        profile_on_exit=False,
        bass_kernel=nc.m,
        offline_processing=True,
        fname="*_body*",
        metadata={},
    )
    return _process_ntff_profile(
        profile, tmpdir, nc, core_ids, None, False, {}, False
    ).as_bass_kernel_results(results)


def run_spmd(inputs, trace=False, mm_dtype="bf16+fp8", tmpdir=None):
    from concourse.bass_utils import run_bass_kernel_spmd

    nc = _get_nc(mm_dtype)
    in_maps = make_in_maps(inputs)
    if trace:
        res = _run_traced(nc, in_maps, list(range(NCORES)), tmpdir=tmpdir)
    else:
        res = run_bass_kernel_spmd(nc, in_maps, list(range(NCORES)), trace=False)
    return gather_output(res.results), res


def kernel(**inputs) -> np.ndarray:
    out, _ = run_spmd(inputs, trace=False, mm_dtype="bf16+fp8")
    return out
